# revision 1
# baseline (speedup 1.0000x reference)
"""Multi-head causal attention (B=2, S=2048, D=1024, H=16, dh=64) on 8 TRN2 cores.

Strategy (81 us baseline -> 53.7 us)
------------------------------------
- Shard the 32 (batch, head) pairs across 8 cores, 4 pairs each (cores 0-3: b=0,
  cores 4-7: b=1). Pure data parallel, no collectives. Two heads are packed per
  128 SBUF partitions (64 dh-rows each).
- Per head, S^T = K @ Q^T on the PE (contraction over dh=64 on the partition
  axis) gives P^T = exp(S^T) directly in [k, q] layout -- no transposes.
- PE time under the cost model is the total of output columns streamed, serial
  per matmul. So P@V runs SWAPPED: P^T [128k x 128q] is the stationary
  operand and V [128k x 65] the moving one, accumulating O[q, 65] per 128-q
  subblock over k-blocks. That streams 65 cols per k-block instead of W<=512,
  using all 128 output partitions (the V-stationary form wastes half the
  PE on a 65-row output). Halves P@V PE time; host also skips a transpose.
- All matmul operands are bf16 (1 cyc/row flat, no fp32r narrow-AP penalty);
  PSUM accumulation stays fp32. One accumulation group per (chunk, head)
  PSUM bank: start=True only on the bank's first matmul (its lazy zero
  covers the whole 2 KB region), stop=True on its last.
- Softmax without max-subtraction (scores are O(1) after the 1/sqrt(dh) scale,
  exp never overflows in fp32; identical result up to fp rounding).
- exp is the bottleneck engine-wise (one ACT at 0.83 ns/col and ~8.7M score
  elements per core), so the score -> P conversion is SPLIT between ACT
  (true exp) and DVE (Schraudolph bit-trick: int16(x*128/ln2 + B)
  reinterpreted as bf16 ~= e^x +-3%; the shared ones-column softmax
  normalization cancels most of it -- measured 6e-3 max rel err end to end
  with a ~50% split, vs 2e-2 tolerance). A static greedy planner balances
  per-engine busy time (cols/rate + per-instruction overheads); narrow
  diagonal blocks fuse both heads' slices into one strided instruction.
- Row sums l_q come free from P@V via a ones-column appended to V; the host
  divides. Causal-mask handling: host-side block planning at [128k x 512q]
  granularity skips all-masked blocks; mixed blocks get a 0/1 multiply on
  the otherwise-idle GPSIMD engine (both heads in one strided instruction)
  from deduplicated bf16 mask tiles.
- The whole (group, chunk, block) stream is software-pipelined flat: P@V
  retires LA=5 blocks behind S^T/exp so the in-order PE queue never stalls
  ready next-chunk matmuls behind exp-waits; PSUM holds 3 score tiles
  (6 banks) + 2 output banks.
- Input DMAs all issue on the SP HWDGE queue (a small leading K/Q slice
  first); output stores go on the GPSIMD SWDGE queue so their copy-waits
  cannot block input prefetches; the drain store splits across two queues.
"""

import os
import sys
from contextlib import ExitStack

import numpy as np

for _p in ("/opt/trn_rl_repo", "/root/.axon_site/_ro/trn_rl_repo"):
    if os.path.isdir(_p) and _p not in sys.path:
        sys.path.insert(0, _p)
        break

import concourse.bacc as bacc  # noqa: E402
import concourse.mybir as mybir  # noqa: E402
import concourse.tile as tile  # noqa: E402
from concourse.bass_utils import run_bass_kernel_spmd  # noqa: E402

F32 = mybir.dt.float32
BF16 = mybir.dt.bfloat16
I16 = mybir.dt.int16
EXP = mybir.ActivationFunctionType.Exp
MULT = mybir.AluOpType.mult
ADD = mybir.AluOpType.add

N_CORES = 8
H = 16
DH = 64
QBLK = 512
KBLK = 128
VW = DH + 1

PV_SWAP = os.environ.get("K_PV_SWAP", "1") == "1"
USE_SCHRAUD = os.environ.get("K_SCHRAUD", "1") == "1"
POOL_MASK = os.environ.get("K_POOL_MASK", "1") == "1"

# Schraudolph constants for bf16: i16 = x * (2^7/ln2) + SCH_B, bitcast bf16.
SCH_A = 128.0 / float(np.log(2.0))
SCH_B = 16249.0

# engine-time model used only for the static exp/copy split (ns)
ACT_RATE, ACT_OVH = 0.833, 0.833 * 222 + 32 + 100 + float(os.environ.get("K_AOVH", "0"))
DVE_RATE, DVE_OVH = 1.042, 1.042 * 120 + 45 + 100

LAST_RESULTS = None  # BassKernelResults of the most recent kernel() call


def _plan_blocks(mask):
    """Classify [KBLK x QBLK] blocks of S^T per q-chunk, union over batch.

    Returns (plans, uniq_contents):
      plans[qc] = list of (kk, c0, c1, m0, m1, uid); block covers k rows
        kk*KBLK..+KBLK and q columns qc*QBLK+c0..qc*QBLK+c1. If uid >= 0,
        multiply P^T block columns [m0, m1) by mask tile `uid`.
      uniq_contents[uid] = float32 [B, KBLK, mw] 0/1 tile (per-batch content).
    """
    B, S, _ = mask.shape
    NQ, NK = S // QBLK, S // KBLK
    uniq_keys = {}
    uniq_contents = []
    plans = []
    for qc in range(NQ):
        out = []
        for kk in range(NK):
            sub = mask[:, qc * QBLK:(qc + 1) * QBLK, kk * KBLK:(kk + 1) * KBLK]
            anyk = sub.any(axis=(0, 2))  # [QBLK] column needed?
            if not anyk.any():
                continue
            c0 = int(anyk.argmax()) & ~3
            c1 = min(QBLK, (QBLK - int(anyk[::-1].argmax()) + 3) & ~3)
            if PV_SWAP:
                # swap-mode P@V slices lhsT at 128-aligned q-subblocks
                c0 &= ~(KBLK - 1)
                c1 = min(QBLK, (c1 + KBLK - 1) & ~(KBLK - 1))
            allk = sub.all(axis=(0, 2))
            dirty = ~allk
            dirty[:c0] = False
            dirty[c1:] = False
            if dirty.any():
                m0 = int(dirty.argmax()) & ~3
                m1 = min(QBLK, (QBLK - int(dirty[::-1].argmax()) + 3) & ~3)
                dirty[m0:m1] = True
                content = np.zeros((B, KBLK, m1 - m0), np.float32)
                for bb in range(B):
                    content[bb] = sub[bb, m0:m1, :].T
                key = content.tobytes()
                uid = uniq_keys.get(key)
                if uid is None:
                    uid = len(uniq_contents)
                    uniq_keys[key] = uid
                    uniq_contents.append(content)
            else:
                m0 = m1 = 0
                uid = -1
            out.append((kk, c0, c1, m0, m1, uid))
        plans.append(out)
    mw = max((c.shape[2] for c in uniq_contents), default=1)
    uniq_padded = []
    for c in uniq_contents:
        p = np.zeros((B, KBLK, mw), np.float32)
        p[:, :, :c.shape[2]] = c
        uniq_padded.append(p)
    return plans, uniq_padded


def _plan_engines(S, n_groups, plans):
    """Greedy-balance the per-block exp work (and out-copies) across ACT/DVE.

    Returns dict keyed (gi, qc, kk, h) -> 'a'|'v' for exp items (h=-1 means
    the fused full-width pair item) plus ('copy', gi, qc, h) -> 'a'|'v'.
    DVE is pre-loaded with the mask-multiply cost it always carries.
    """
    NQ = S // QBLK
    load = {"a": 0.0, "v": 0.0}
    assign = {}

    items = []
    for gi in range(n_groups):
        for qc in range(NQ):
            for (kk, c0, c1, m0, m1, uid) in plans[qc]:
                W = c1 - c0
                items.append(((gi, qc, kk, -1), 2 * W))
            ccols = 4 * VW if PV_SWAP else QBLK
            for h in range(2):
                items.append((("copy", gi, qc, h), ccols))

    cmode = os.environ.get("K_COPY", "g")
    seed = int(os.environ.get("K_SEED", "10"))
    for ii, (key, cols) in enumerate(items):
        jit = ((hash((ii, seed)) % 101) - 50) if seed else 0
        if key[0] == "copy" and cmode != "g":
            assign[key] = cmode
            load[cmode] += (ACT_RATE if cmode == "a" else DVE_RATE) * cols \
                + (ACT_OVH if cmode == "a" else DVE_OVH)
            continue
        ta = load["a"] + ACT_RATE * cols + ACT_OVH + jit
        tv = load["v"] + DVE_RATE * cols + DVE_OVH
        if ta <= tv:
            assign[key] = "a"
            load["a"] = ta
        else:
            assign[key] = "v"
            load["v"] = tv
    return assign, load


def _build(S, n_groups, n_pairs, plans, n_uniq, mw=1, repeat=1,
           la=None, p_bufs=None, s_bufs=None, o_bufs=None, osb_bufs=None):
    if la is None:
        la = int(os.environ.get("K_LA", "5"))
    if p_bufs is None:
        p_bufs = int(os.environ.get("K_PBUFS", "6"))
    if osb_bufs is None:
        osb_bufs = int(os.environ.get("K_OSB", "4"))
    if s_bufs is None:
        s_bufs = int(os.environ.get("K_SBUFS", "3"))
    if o_bufs is None:
        o_bufs = int(os.environ.get("K_OBUFS", "1"))
    """Build the single SPMD program run identically on all cores."""
    NQ, NK = S // QBLK, S // KBLK
    nc = bacc.Bacc("TRN2", target_bir_lowering=False, debug=False)
    qt = nc.declare_dram_parameter("qt", [n_groups, 128, S], BF16, isOutput=False)
    kt = nc.declare_dram_parameter("kt", [n_groups, 128, S], BF16, isOutput=False)
    vv = nc.declare_dram_parameter("vv", [n_pairs, 128, NK * VW], BF16,
                                   isOutput=False)
    mk = nc.declare_dram_parameter("mk", [max(n_uniq, 1), 128, 2 * mw], BF16,
                                   isOutput=False)
    if PV_SWAP:
        ot = nc.declare_dram_parameter("ot", [n_groups, NQ, 128, 8 * VW], F32,
                                       isOutput=True)
    else:
        ot = nc.declare_dram_parameter("ot", [n_pairs, VW, S], F32, isOutput=True)

    engplan, _ = _plan_engines(S, n_groups, plans)

    with tile.TileContext(nc) as tc, ExitStack() as ctx:
        qpool = ctx.enter_context(tc.tile_pool(name="qpool", bufs=2))
        kpool = ctx.enter_context(tc.tile_pool(name="kpool", bufs=2))
        vpool = ctx.enter_context(tc.tile_pool(name="vpool", bufs=3))
        mpool = ctx.enter_context(tc.tile_pool(name="mpool", bufs=1))
        ppool = ctx.enter_context(tc.tile_pool(name="ppool", bufs=p_bufs))
        obuf = ctx.enter_context(tc.tile_pool(name="obuf", bufs=osb_bufs))
        spool = ctx.enter_context(tc.tile_pool(name="spool", bufs=s_bufs, space="PSUM"))
        opool = ctx.enter_context(tc.tile_pool(name="opool", bufs=2, space="PSUM"))

        warm = mpool.tile([128, 8], F32)
        warmb = mpool.tile([8, 8], BF16)
        mtile = mpool.tile([128, max(n_uniq, 1) * 2 * mw], BF16)

        def exp_emit(eng, dst, src):
            if eng == "a" or not USE_SCHRAUD:
                nc.scalar.activation(dst, src, EXP)
            else:
                nc.vector.tensor_scalar(dst.bitcast(I16), src, SCH_A, SCH_B,
                                        MULT, ADD)

        giter = [g for _ in range(repeat) for g in range(n_groups)]
        # flatten (group, chunk, block) into one stream so the P@V retire
        # lag (LA) pipelines ACROSS chunk and group boundaries: the PE queue
        # is in-order, so a chunk drain emitted before the next chunk's
        # S-matmuls would stall ready work behind exp-waits
        flat = []
        spread = os.environ.get("K_SPREAD", "0") == "1"
        lastord = os.environ.get("K_LASTORD", "0123")
        for gi, g in enumerate(giter):
            qorder = list(range(NQ))
            if gi == len(giter) - 1 and len(lastord) == NQ:
                qorder = [int(c) for c in lastord]
            for qc in qorder:
                blocks = plans[qc]
                if spread and PV_SWAP and len(blocks) > 5:
                    # interleave the (masked, narrow) diagonal blocks among
                    # the full ones so their exp->Pool-mask->P@V chains don't
                    # bunch up at the chunk end (any order is safe in swap
                    # mode: accumulation start/stop is per output bank)
                    fulls = [b for b in blocks if b[5] < 0]
                    diags = [b for b in blocks if b[5] >= 0]
                    merged = []
                    fi = di = 0
                    for t in range(len(blocks)):
                        ff = (fi + 1) / (len(fulls) + 1) if fulls else 2
                        dd = (di + 1) / (len(diags) + 1) if diags else 2
                        if fi < len(fulls) and (di >= len(diags) or ff <= dd):
                            merged.append(fulls[fi]); fi += 1
                        else:
                            merged.append(diags[di]); di += 1
                    blocks = merged
                for bi, blk in enumerate(blocks):
                    flat.append((gi, g, qc, bi, blk, len(blocks)))
        N = len(flat)
        LA = la
        gtiles = {}
        cstate = {}
        staged = []

        def emit_group_loads(gi, g):
            ktile = kpool.tile([128, S], BF16, tag="kt")
            qtile = qpool.tile([128, S], BF16, tag="qt")
            vtiles = [vpool.tile([128, NK * VW], BF16, tag=f"vt{h}",
                                 name=f"vt{h}") for h in range(2)]
            # first-needed-first: a small leading K/Q slice unblocks the
            # opening S-matmuls (split across two idle DGE queues at t=0),
            # then the bulk follows in one transfer each
            if gi == 0:
                nc.scalar.dma_start(qtile[:, 0:QBLK], qt[g, :, 0:QBLK])
                nc.sync.dma_start(ktile[:, 0:KBLK], kt[g, :, 0:KBLK])
                # ACT exp-table load overlaps the initial input DMAs
                nc.vector.memset(warm[:], 0.0)
                nc.scalar.activation(warm[:], warm[:], EXP)
                for u in range(n_uniq):
                    nc.gpsimd.dma_start(mtile[:, u * 2 * mw:(u + 1) * 2 * mw],
                                        mk[u])
            else:
                nc.sync.dma_start(ktile[:, 0:KBLK], kt[g, :, 0:KBLK])
                nc.sync.dma_start(qtile[:, 0:QBLK], qt[g, :, 0:QBLK])
            # interleave the bulk K/Q/V loads in chunk-sized pieces ordered
            # by first use -- the transfer device is serial, and a monolithic
            # K bulk would push chunk 1's Q slice past its consumption time
            nc.sync.dma_start(ktile[:, KBLK:QBLK], kt[g, :, KBLK:QBLK])
            nq4 = (NK // 4) * VW
            for h in range(2):
                nc.sync.dma_start(vtiles[h][:, 0:nq4], vv[2 * g + h, :, 0:nq4])
            for c0 in range(QBLK, S, QBLK):
                nc.sync.dma_start(qtile[:, c0:c0 + QBLK], qt[g, :, c0:c0 + QBLK])
                nc.sync.dma_start(ktile[:, c0:c0 + QBLK], kt[g, :, c0:c0 + QBLK])
                v1 = min(c0 // QBLK * nq4 + nq4, NK * VW)
                for h in range(2):
                    nc.sync.dma_start(vtiles[h][:, c0 // QBLK * nq4:v1],
                                      vv[2 * g + h, :, c0 // QBLK * nq4:v1])
            gtiles[gi] = (ktile, qtile, vtiles)

        # retire lag per block: the first blocks of each chunk wait extra
        # fills so the previous chunk's PSUM-bank copy (WAR on o_bufs=1)
        # completes before their start=True matmul needs the bank
        xlag = int(os.environ.get("K_XLAG", "2"))
        lag = [LA + (xlag if flat[r][3] < 2 else 0) for r in range(N)]
        rptr = 0
        for idx in range(N + LA + xlag):
            if idx < N:
                gi, g, qc, bi, blk, nb = flat[idx]
                if gi not in gtiles:
                    emit_group_loads(gi, g)
                ktile, qtile, _ = gtiles[gi]
                kk, c0, c1, m0, m1, uid = blk
                W = c1 - c0
                s_ps = spool.tile([128, 2 * QBLK], F32, tag="s")
                p_t = ppool.tile([128, 2 * QBLK], BF16, tag="p")
                q0 = qc * QBLK + c0
                for h in range(2):
                    nc.tensor.matmul(
                        s_ps[:, h * QBLK + c0:h * QBLK + c1],
                        lhsT=ktile[64 * h:64 * h + 64,
                                   kk * KBLK:(kk + 1) * KBLK],
                        rhs=qtile[64 * h:64 * h + 64, q0:q0 + W],
                        start=True, stop=True)
                eng = engplan[(g, qc, kk, -1)]
                if W == QBLK:
                    if idx == N - 1:
                        # drain tail: halve the final exp latency by
                        # splitting it across both engines
                        e0 = os.environ.get("K_DR", "av")
                        exp_emit(e0[0], p_t[:, 0:QBLK], s_ps[:, 0:QBLK])
                        exp_emit(e0[1], p_t[:, QBLK:2 * QBLK],
                                 s_ps[:, QBLK:2 * QBLK])
                    else:
                        exp_emit(eng, p_t[:, 0:2 * QBLK], s_ps[:, 0:2 * QBLK])
                else:
                    # one strided instruction covers both heads'
                    # [c0, c1) slices (head stride QBLK)
                    sv = s_ps[:].rearrange("p (a q) -> p a q", a=2)
                    pv = p_t[:].rearrange("p (a q) -> p a q", a=2)
                    exp_emit(eng, pv[:, :, c0:c1], sv[:, :, c0:c1])
                if uid >= 0:
                    # both heads in one strided Pool multiply; mask tile
                    # content is duplicated per head halves
                    pm = p_t[:].rearrange("p (a q) -> p a q", a=2)
                    pm = pm[:, :, m0:m1]
                    mm = mtile[:, uid * 2 * mw:uid * 2 * mw + 2 * (m1 - m0)]
                    mm = mm.rearrange("p (a b) -> p a b", a=2)
                    if POOL_MASK:
                        nc.gpsimd.tensor_mul(pm, pm, mm)
                    else:
                        nc.vector.tensor_mul(pm, pm, mm)
                staged.append((flat[idx], p_t))
            while rptr < N and idx - rptr >= lag[rptr]:
                r = rptr
                rptr += 1
                (gi, g, qc, bi, blk, nb), p_t = staged[r]
                kk, c0, c1, m0, m1, uid = blk
                W = c1 - c0
                vtiles = gtiles[gi][2]
                st = cstate.get((gi, qc))
                if st is None:
                    if PV_SWAP:
                        o_ps = [opool.tile([128, 4 * VW], F32, tag=f"o{h}",
                                           name=f"o_ps{h}", bufs=o_bufs)
                                for h in range(2)]
                        # one accumulation group per (chunk, head) PSUM
                        # bank: start=True only on the bank's first matmul
                        # (its lazy-zero covers the whole 2 KB region, so
                        # later subblock regions accumulate onto zero),
                        # stop=True on the bank's last matmul
                        n_pv = sum((b[2] - b[1]) // KBLK for b in plans[qc])
                    else:
                        o_ps = [opool.tile([VW, QBLK], F32, tag=f"o{h}",
                                           name=f"o_ps{h}", bufs=o_bufs)
                                for h in range(2)]
                        n_pv = 0
                    st = {"o": o_ps, "n": n_pv, "c": [0, 0]}
                    cstate[(gi, qc)] = st
                o_ps = st["o"]
                if PV_SWAP:
                    for h in range(2):
                        for jj in range(c0 // KBLK, (c1 + KBLK - 1) // KBLK):
                            je = min((jj + 1) * KBLK, c1)
                            M = je - jj * KBLK
                            st["c"][h] += 1
                            nc.tensor.matmul(
                                o_ps[h][0:M, jj * VW:(jj + 1) * VW],
                                lhsT=p_t[:, h * QBLK + jj * KBLK:
                                         h * QBLK + je],
                                rhs=vtiles[h][:, kk * VW:(kk + 1) * VW],
                                start=st["c"][h] == 1,
                                stop=st["c"][h] == st["n"])
                else:
                    for h in range(2):
                        nc.tensor.matmul(
                            o_ps[h][:, c0:c1],
                            lhsT=vtiles[h][:, kk * VW:(kk + 1) * VW],
                            rhs=p_t[:, h * QBLK + c0:h * QBLK + c1],
                            start=(bi == 0), stop=(bi == nb - 1))
                if bi == nb - 1:
                    del cstate[(gi, qc)]
                    is_final = idx == N + LA - 1
                    if PV_SWAP:
                        osb = obuf.tile([128, 8 * VW], F32, tag="osb")
                        for h in range(2):
                            eng = engplan[("copy", g, qc, h)]
                            dsl = osb[:, h * 4 * VW:(h + 1) * 4 * VW]
                            if eng == "a":
                                nc.scalar.copy(dsl, o_ps[h][:])
                            else:
                                nc.vector.tensor_copy(dsl, o_ps[h][:])
                        dst = ot[g, qc]
                        if is_final:
                            # drain: split the final store over two DGEs
                            nc.sync.dma_start(dst[:, 0:4 * VW],
                                              osb[:, 0:4 * VW])
                            nc.scalar.dma_start(dst[:, 4 * VW:],
                                                osb[:, 4 * VW:])
                        else:
                            # SWDGE queue: an out-store waiting on its copy
                            # must not block the SP queue head (input
                            # prefetches flow behind it)
                            nc.gpsimd.dma_start(dst, osb[:])
                    else:
                        for h in range(2):
                            eng = engplan[("copy", g, qc, h)]
                            dst = ot[2 * g + h, :, qc * QBLK:(qc + 1) * QBLK]
                            osb = obuf.tile([VW, QBLK], F32, tag="osb")
                            if eng == "a":
                                nc.scalar.copy(osb[:], o_ps[h][:])
                            else:
                                nc.vector.tensor_copy(osb[:], o_ps[h][:])
                            if is_final:
                                hw = QBLK // 2
                                nc.sync.dma_start(dst[:, 0:hw], osb[:, 0:hw])
                                nc.scalar.dma_start(dst[:, hw:], osb[:, hw:])
                            else:
                                nc.gpsimd.dma_start(dst, osb[:])
    nc.finalize()
    return nc


def _make_in_maps(q4, k4, v4, maskb, uniq, n_groups, per_core):
    B, S = q4.shape[0], q4.shape[1]
    NK = S // KBLK
    n_uniq = len(uniq)
    mw = uniq[0].shape[2] if uniq else 1
    in_maps = []
    for c in range(N_CORES):
        qt = np.empty((n_groups, 128, S), np.float32)
        kt = np.empty((n_groups, 128, S), np.float32)
        vvv = np.empty((per_core, 128, NK * VW), np.float32)
        bs = []
        for lp in range(per_core):
            gp = c * per_core + lp
            b, h = divmod(gp, H)
            bs.append(b)
            g, half = divmod(lp, 2)
            qt[g, 64 * half:64 * half + 64] = q4[b, :, h, :].T
            kt[g, 64 * half:64 * half + 64] = k4[b, :, h, :].T
            vt = np.ones((128, NK, VW), np.float32)
            vt[:, :, :DH] = v4[b, :, h, :].reshape(NK, KBLK, DH).transpose(1, 0, 2)
            vvv[lp] = vt.reshape(128, NK * VW)
        if n_uniq:
            assert len(set(bs)) == 1, "mask tiles assume one batch per core"
            mkarr = np.zeros((n_uniq, 128, 2 * mw), np.float32)
            for u in range(n_uniq):
                mkarr[u, :, 0:mw] = uniq[u][bs[0]]
                mkarr[u, :, mw:2 * mw] = uniq[u][bs[0]]
        else:
            mkarr = np.zeros((1, 128, 2), np.float32)
        import ml_dtypes
        in_maps.append({
            "qt": qt.astype(ml_dtypes.bfloat16),
            "kt": kt.astype(ml_dtypes.bfloat16),
            "vv": vvv.astype(ml_dtypes.bfloat16),
            "mk": mkarr.astype(ml_dtypes.bfloat16),
        })
    return in_maps


def _assemble(results, B, S, per_core):
    D = H * DH
    out = np.empty((B, S, D), np.float32)
    for c in range(N_CORES):
        otc = results[c]["ot"]
        for lp in range(per_core):
            gp = c * per_core + lp
            b, h = divmod(gp, H)
            if PV_SWAP:
                g, half = divmod(lp, 2)
                # otc: [n_groups, NQ, 128, 2 (head), 4 (sub), VW]
                o = otc[g].reshape(S // QBLK, 128, 2, 4, VW)[:, :, half]
                o = o.transpose(0, 2, 1, 3).reshape(S, VW).astype(np.float64)
                l = o[:, DH]
                l = np.where(l == 0.0, 1.0, l)
                out[b, :, h * DH:(h + 1) * DH] = \
                    (o[:, :DH] / l[:, None]).astype(np.float32)
            else:
                l = otc[lp, DH].astype(np.float64)
                l = np.where(l == 0.0, 1.0, l)
                out[b, :, h * DH:(h + 1) * DH] = \
                    (otc[lp, :DH] / l).T.astype(np.float32)
    return out


def kernel(queries, keys, values, mask):
    B, S, D = queries.shape
    assert D == H * DH
    q4 = (np.ascontiguousarray(queries, dtype=np.float32) * 0.125) \
        .reshape(B, S, H, DH)
    k4 = np.ascontiguousarray(keys, dtype=np.float32).reshape(B, S, H, DH)
    v4 = np.ascontiguousarray(values, dtype=np.float32).reshape(B, S, H, DH)
    maskb = np.asarray(mask).astype(bool)

    plans, uniq = _plan_blocks(maskb)
    per_core = (B * H) // N_CORES
    n_groups = per_core // 2

    mw = uniq[0].shape[2] if uniq else 1
    nc = _build(S, n_groups, per_core, plans, len(uniq), mw=mw)
    in_maps = _make_in_maps(q4, k4, v4, maskb, uniq, n_groups, per_core)
    try:
        res = run_bass_kernel_spmd(nc, in_maps, core_ids=list(range(N_CORES)))
    except ModuleNotFoundError:
        os.environ["BASS_NEVER_TRACE"] = "1"
        res = run_bass_kernel_spmd(nc, in_maps, core_ids=list(range(N_CORES)))
    global LAST_RESULTS
    LAST_RESULTS = res
    return _assemble(res.results, B, S, per_core)

